# revision 41
# baseline (speedup 1.0000x reference)
"""GQA attention (B=2, S=2048, dm=1024, 16 Q heads / 4 KV heads, dh=64, RoPE,
causal) on 8 trn2 NeuronCores.

Sharding: core c = (b, g) with b = c // 4 (batch), g = c % 4 (KV group).
Each core computes its 4 Q heads + 1 KV head end-to-end (flash-style) plus its
partial Wo projection; host sums the 4 partials per batch element.

v4 (on top of v3's bf16 matmuls, transposed layouts, denominator-via-ones
rows, filler-interleaved projections):
  - K|V fused projection: one [128,(K|V)] stationary produces K^T and V^T
    in a single accumulation chain (halves the KV projection matmul time).
  - RoPE via DVE stream_shuffle: the head-dim order of Wq/Wk columns is
    permuted host-side so rotate-half becomes an intra-32-partition
    shuffle (+16 mod 32); signs folded into the sinm table. Rope is
    3 DVE ops + 1 GpSimd add, no ScalarE involvement.
  - exp table-set load triggered at t=0, before the bulk constant DMAs.
  - startup DMAs ki-interleaved across both HWDGE queues so the first
    projection matmuls start ~2us after DMA begins.
  - output leaves the core as bf16 (host upcasts), halving out-DMA.
  - tail: last-chunk denominator staging + half the final Wo casts run on
    the (by then idle) ScalarE instead of the DVE.
  - V tiles transposed by the DMA XBAR (SBUF->SBUF) instead of
    PE-transpose + PSUM staging + DVE copy: decongests the pa PSUM ring,
    which was the hidden source of run-to-run jitter and HAM re-throttles.

NOTE (measured on hw): fp8/DoubleRow for the P@V chain and Schraudolph
exp-on-DVE are numerically DEAD ENDS here -- attention output on iid random
data is a cancelling average, so per-prob noise (fp8 ~2.5-3.6% per element)
lands ~1:1 in the output norm-rel-err, busting the 2e-2 gate. The whole
P/V/score chain must stay bf16.
"""

import os
import sys
from collections import deque

import numpy as np
import ml_dtypes

try:
    from concourse import bass_utils
except ImportError:
    for _p in ("/opt/trn_rl_repo", "/root/.axon_site/_ro/trn_rl_repo"):
        if os.path.isdir(_p) and _p not in sys.path:
            sys.path.insert(0, _p)
    from concourse import bass_utils

import concourse.bass as bass
import concourse.mybir as mybir
import concourse.tile as tile
from concourse import bacc

F32 = mybir.dt.float32
BF = mybir.dt.bfloat16
EXP = mybir.ActivationFunctionType.Exp
COPY = mybir.ActivationFunctionType.Copy
MULT = mybir.AluOpType.mult
ADD = mybir.AluOpType.add

B, S, DM = 2, 2048, 1024
H, KV, DH = 16, 4, 64
HPG = H // KV          # 4 q-heads per kv group (per core)
DG = HPG * DH          # 256 local q dims
NCHIPS = 8
QB = 512               # q block width
KT = 128               # k tile width
NKT = S // KT          # 16 k tiles
NCH = S // QB          # 4 chunks == 4 q blocks

BF_NP = ml_dtypes.bfloat16

# rotate-half as intra-quadrant shuffle: out[i] = in[(i+16)%32] per 32-block
SHUF16 = [(i + 16) % 32 for i in range(32)]


def build_bass():
    nc = bacc.Bacc()
    xcc = nc.declare_dram_parameter("xcc", [NCH, 128, 8, QB], BF, isOutput=False)
    wq = nc.declare_dram_parameter("wq", [128, 8, DG], BF, isOutput=False)
    wkv = nc.declare_dram_parameter("wkv", [128, 8, 128], BF, isOutput=False)
    wo = nc.declare_dram_parameter("wo", [128, 2, DM], BF, isOutput=False)
    cos2 = nc.declare_dram_parameter("cos2", [128, S], BF, isOutput=False)
    sinm2 = nc.declare_dram_parameter("sinm2", [128, S], BF, isOutput=False)
    ident = nc.declare_dram_parameter("ident", [128, 128], BF, isOutput=False)
    mask2 = nc.declare_dram_parameter("mask2", [128, 2, 128], BF, isOutput=False)
    out = nc.declare_dram_parameter("out", [S, DM], BF, isOutput=True)

    from contextlib import ExitStack
    with tile.TileContext(nc) as tc, ExitStack() as es:
        cst = es.enter_context(tc.tile_pool(name="cst", bufs=1))
        sbQ = es.enter_context(tc.tile_pool(name="sbQ", bufs=1))
        sbX = es.enter_context(tc.tile_pool(name="sbX", bufs=2))
        sbR = es.enter_context(tc.tile_pool(name="sbR", bufs=3))
        sbP = es.enter_context(tc.tile_pool(name="sbP", bufs=6))
        sbN = es.enter_context(tc.tile_pool(name="sbN", bufs=3))
        sbO = es.enter_context(tc.tile_pool(name="sbO", bufs=6))
        ps = es.enter_context(tc.tile_pool(name="ps", bufs=2, space="PSUM"))

        xcs = [sbX.tile([128, 8, QB], BF, tag="xc", name=f"xc{i}")
               for i in range(NCH)]
        # trigger the exp table-set load (~2.7us) before the scalar queue
        # fills with bulk constant DMAs; input is a freshly-memset scratch
        wu_l = cst.tile([128, 128], BF)
        nc.gpsimd.memset(wu_l[:], 0.5)
        wu_act = cst.tile([128, 8], BF)
        nc.scalar.activation(wu_act[:], wu_l[:, 0:8], EXP, scale=0.01)
        # startup DMA: x pieces on the sync queue, weights + tables on the
        # scalar queue, ki-interleaved so the first projection matmuls can
        # start ~2us after DMA begins instead of waiting for 1.5MB
        wq_sb = cst.tile([128, 8, DG], BF)
        wkv_sb = cst.tile([128, 8, 128], BF)
        cos_sb = cst.tile([128, S], BF)
        sinm_sb = cst.tile([128, S], BF)
        nc.scalar.dma_start(wkv_sb[:], wkv[:])
        for k in range(4):
            nc.sync.dma_start(xcs[0][:, 2 * k:2 * k + 2, :],
                              xcc[0][:, 2 * k:2 * k + 2, :])
            nc.scalar.dma_start(wq_sb[:, 2 * k:2 * k + 2, :],
                                wq[:, 2 * k:2 * k + 2, :])
        nc.scalar.dma_start(cos_sb[:, 0:QB], cos2[:, 0:QB])
        nc.scalar.dma_start(sinm_sb[:, 0:QB], sinm2[:, 0:QB])
        nc.sync.dma_start(xcs[1][:], xcc[1][:])
        id_sb = cst.tile([128, 128], BF)
        nc.scalar.dma_start(id_sb[:], ident[:])
        mask_sb = cst.tile([128, 2, 128], BF)
        nc.scalar.dma_start(mask_sb[:], mask2[:])
        nc.sync.dma_start(xcs[2][:], xcc[2][:])
        nc.scalar.dma_start(cos_sb[:, QB:2 * QB], cos2[:, QB:2 * QB])
        nc.scalar.dma_start(sinm_sb[:, QB:2 * QB], sinm2[:, QB:2 * QB])
        nc.scalar.dma_start(cos_sb[:, 2 * QB:3 * QB], cos2[:, 2 * QB:3 * QB])
        nc.scalar.dma_start(sinm_sb[:, 2 * QB:3 * QB], sinm2[:, 2 * QB:3 * QB])
        wo_sb = cst.tile([128, 2, DM], BF)

        # HAM warm-up: keep the PE busy on scratch data while the first
        # DMAs land, so chunk-0 projections run at 2.4 GHz, not 1.2.
        wu_r = cst.tile([128, QB], BF)
        nc.gpsimd.memset(wu_r[:], 0.5)
        wu_ps = ps.tile([128, QB], F32, tag="cp", name="wu_ps")
        for _ in range(24):
            nc.tensor.matmul(wu_ps[:, 0:128], wu_l[:], wu_r[:, 0:128],
                             start=True, stop=True)

        # persistent activations
        qT2 = sbQ.tile([128, 2, S], BF)              # Q^T, head pairs stacked
        kT = sbQ.tile([128, S], BF)                  # K^T duplicated rows
        vt = [sbQ.tile([128, 128], BF, tag=f"v{i}", name=f"v{i}")
              for i in range(NKT)]
        for i in range(NKT):
            nc.gpsimd.memset(vt[i][:, DH:128], 1.0)  # denominator columns
        packed = sbQ.tile([128, 2, S], BF)           # normalized ctx^T

        def rope(dsts, qp, rows, c0):
            """dst (bf16) = qp[0:rows]*cos + shuffle16(qp[0:rows])*sinm.

            qp is fp32 PSUM; rows is 64 or 128 (1 or 2 stacked heads).
            Head-dim order is host-permuted so rotate-half is the intra-
            quadrant +16 shuffle with signs folded into sinm. DVE does
            shuffle + 2 multiplies, GpSimdE the final add.
            """
            rot = sbR.tile([128, QB], F32, tag="rot")
            nc.vector.stream_shuffle(rot[0:rows, :], qp[0:rows, :], SHUF16)
            t1 = sbR.tile([128, QB], BF, tag="t1")
            nc.vector.tensor_tensor(t1[0:rows, :], qp[0:rows, :],
                                    cos_sb[0:rows, c0:c0 + QB], MULT)
            rs = sbR.tile([128, QB], BF, tag="rs")
            nc.vector.tensor_tensor(rs[0:rows, :], rot[0:rows, :],
                                    sinm_sb[0:rows, c0:c0 + QB], MULT)
            for d, lo, nr in dsts:
                nc.gpsimd.tensor_tensor(
                    d, t1[lo:lo + nr, :], rs[lo:lo + nr, :], ADD)

        def proj_fillers(ch):
            """Closures emitting the projections+rope for chunk ch."""
            c0 = ch * QB
            xc = xcs[ch]
            fl = []
            state = {}

            def qchain(mt, lo, hi):
                def go():
                    if lo == 0:
                        state[mt] = ps.tile([128, QB], F32, tag="pa",
                                            name=f"qp{ch}{mt}", uniquify=True)
                    qp = state[mt]
                    for ki in range(lo, hi):
                        nc.tensor.matmul(
                            qp[:], wq_sb[:, ki, mt * 128:mt * 128 + 128],
                            xc[:, ki, :], start=(ki == 0), stop=(ki == 7))
                return go

            def qrope(mt):
                def go():
                    rope([(qT2[:, mt, c0:c0 + QB], 0, 128)], state[mt], 128, c0)
                return go

            for lo in range(0, 8, 2):
                fl.append(qchain(0, lo, min(lo + 2, 8)))
            fl.append(qrope(0))

            def kvchain(lo, hi):
                def go():
                    if lo == 0:
                        state['kv'] = ps.tile([128, QB], F32, tag="pa",
                                              name=f"kvp{ch}", uniquify=True)
                    kvp = state['kv']
                    for ki in range(lo, hi):
                        nc.tensor.matmul(kvp[:], wkv_sb[:, ki, :],
                                         xc[:, ki, :],
                                         start=(ki == 0), stop=(ki == 7))
                return go

            for lo in range(0, 8, 2):
                fl.append(kvchain(lo, min(lo + 2, 8)))

            def krope():
                rope([(kT[0:DH, c0:c0 + QB], 0, DH)], state['kv'], DH, c0)
                # duplicate K^T rows for the second concurrent head matmul
                nc.sync.dma_start(kT[DH:128, c0:c0 + QB], kT[0:DH, c0:c0 + QB])
            fl.append(krope)

            def vmove():
                vTs = sbR.tile([DH, QB], BF, tag="vT")
                nc.vector.tensor_copy(vTs[:], state['kv'][DH:128, :])
                state['vT'] = vTs
            fl.append(vmove)

            def vtrans(sub):
                def go():
                    # DMA XBAR transpose SBUF->SBUF: no PE pass, no DVE copy,
                    # and no tp allocation in the contended pa PSUM ring
                    nc.sync.dma_start(
                        vt[ch * 4 + sub][:, 0:DH],
                        state['vT'][:, sub * 128:sub * 128 + 128],
                        transpose=True)
                return go

            for sub in range(QB // 128):
                fl.append(vtrans(sub))
            for lo in range(0, 8, 2):
                fl.append(qchain(1, lo, min(lo + 2, 8)))
            fl.append(qrope(1))
            return fl

        def wo_fillers(jb, tail=False):
            """Closures emitting the Wo projection for q block jb. In the
            tail, ScalarE is idle: alternate the PSUM->SBUF casts onto it so
            the DVE does not serialize the endgame."""
            q0 = jb * QB
            fl = []

            def wostep(st, nb):
                def go():
                    s0 = q0 + st * 128
                    op = ps.tile([128, QB], F32, tag="pa",
                                 name=f"op{jb}{st}{nb}", uniquify=True)
                    for kt2 in range(2):
                        nc.tensor.matmul(
                            op[:], packed[:, kt2, s0:s0 + 128],
                            wo_sb[:, kt2, nb * QB:nb * QB + QB],
                            start=(kt2 == 0), stop=(kt2 == 1))
                    ot = sbO.tile([128, QB], BF, tag="ot")
                    if tail and (st + nb) % 2 == 0:
                        nc.scalar.activation(ot[:], op[:], COPY)
                    else:
                        nc.vector.tensor_copy(ot[:], op[:])
                    # tail: drain the last block's output on both queues
                    dq = nc.scalar if tail and nb == 1 else nc.sync
                    dq.dma_start(
                        out[s0:s0 + 128, nb * QB:nb * QB + QB], ot[:])
                return go

            for st in range(QB // 128):
                for nb in range(2):
                    fl.append(wostep(st, nb))
            return fl

        fillers = deque()          # entries: (deadline_chunk_or_None, fn)
        dummy_budget = [32]

        def drain(n):
            ran = False
            for _ in range(n):
                if not fillers:
                    break
                fillers.popleft()[1]()
                ran = True
            if not ran and dummy_budget[0] > 0:
                # keep-warm: short scratch matmul so the HAM clock gate does
                # not re-throttle during scalar-paced stretches
                dummy_budget[0] -= 1
                dp = ps.tile([128, QB], F32, tag="pa", name="dum",
                             uniquify=True)
                nc.tensor.matmul(dp[:, 0:256], wu_l[:], wu_r[:, 0:256],
                                 start=True, stop=True)

        # chunks 0 and 1 project upfront: DVE/GpSimdE are idle here, so
        # their rope chains cost nothing and attention never waits on them
        for f in proj_fillers(0):
            f()
        for f in proj_fillers(1):
            f()

        for ch in range(NCH):
            jb = ch
            q0 = jb * QB
            nkt = 4 * (jb + 1)
            # just-in-time bulk DMAs, then queue projections + previous Wo
            if ch == 0:
                nc.scalar.dma_start(wo_sb[:], wo[:])
                nc.sync.dma_start(xcs[3][:], xcc[3][:])
                nc.scalar.dma_start(cos_sb[:, 3 * QB:], cos2[:, 3 * QB:])
                nc.scalar.dma_start(sinm_sb[:, 3 * QB:], sinm2[:, 3 * QB:])
            # force only overdue projections (normally none: the per-step
            # drain spreads them); never burst the whole queue
            while fillers and fillers[0][0] is not None and fillers[0][0] <= ch:
                fillers.popleft()[1]()
            if ch + 2 < NCH:
                fillers.extend((ch + 2, f) for f in proj_fillers(ch + 2))
            if ch >= 1:
                fillers.extend((None, f) for f in wo_fillers(ch - 1))
            steps_left = 2 * nkt
            prefetch = {}
            for pair in range(2):
                cps = [ps.tile([128, QB], F32, tag="cp", name=f"cp{jb}{pair}{hh}")
                       for hh in range(2)]

                def score_step(kt_i):
                    """scores + exp + mask for one k tile (one step ahead of
                    the ctx matmuls so the PE never sits behind ScalarE)."""
                    cc = max(0, (kt_i - 4 * jb) * KT)
                    sp = ps.tile([128, 2, QB], F32, tag="sp", name="sp",
                                 uniquify=True)
                    for hh in range(2):
                        p0 = hh * DH
                        nc.tensor.matmul(
                            sp[:, hh, cc:],
                            kT[p0:p0 + DH, kt_i * KT:kt_i * KT + KT],
                            qT2[p0:p0 + DH, pair, q0 + cc:q0 + QB],
                            start=True, stop=True)
                    pt = sbP.tile([128, 2, QB], BF, tag="pt", name="pt",
                                  uniquify=True)
                    nc.scalar.activation(pt[:, :, cc:], sp[:, :, cc:], EXP,
                                         scale=0.125)
                    if kt_i >= 4 * jb:
                        nc.vector.tensor_tensor(
                            pt[:, :, cc:cc + KT], pt[:, :, cc:cc + KT],
                            mask_sb[:], MULT)
                    return pt

                pts = dict(prefetch)
                prefetch.clear()
                if 0 not in pts:
                    pts[0] = score_step(0)
                for kt_i in range(nkt):
                    if kt_i + 1 < nkt and kt_i + 1 not in pts:
                        pts[kt_i + 1] = score_step(kt_i + 1)
                    cc = max(0, (kt_i - 4 * jb) * KT)
                    pt = pts.pop(kt_i)
                    for hh in range(2):
                        nc.tensor.matmul(
                            cps[hh][:, cc:], vt[kt_i][:], pt[:, hh, cc:],
                            start=(kt_i == 0), stop=(kt_i == nkt - 1))
                    # keep the PE busy while ScalarE runs exp
                    drain(min(2, -(-len(fillers) // max(1, steps_left))))
                    steps_left -= 1
                if pair == 1:
                    # flush fillers first: their rope/copy ops must not queue
                    # behind the normalize chain on the DVE
                    drain(len(fillers))
                elif ch == NCH - 1:
                    # last chunk: issue pair 1's first scores+exps before the
                    # normalize chain occupies the engine queues
                    pair = 1
                    prefetch[0] = score_step(0)
                    prefetch[1] = score_step(1)
                    pair = 0
                # normalize: 1/denom (rows 64:128) * ctx (rows 0:64);
                # in the tail ScalarE is idle, so stage denominators there
                for hh in range(2):
                    h = 2 * pair + hh
                    dns = sbN.tile([DH, QB], F32, tag="dns")
                    if ch == NCH - 1:
                        nc.scalar.activation(dns[:], cps[hh][DH:128, :], COPY)
                    else:
                        nc.vector.tensor_copy(dns[:], cps[hh][DH:128, :])
                    rb = sbN.tile([DH, QB], F32, tag="rb")
                    nc.vector.reciprocal_approx_fast(rb[:], dns[:])
                    p0 = (h % 2) * DH
                    nc.vector.tensor_tensor(
                        packed[p0:p0 + DH, h // 2, q0:q0 + QB],
                        cps[hh][0:DH, :], rb[:], MULT)
            drain(len(fillers))
        for f in wo_fillers(NCH - 1, tail=True):
            f()
    nc.compile()
    return nc


# within-head row permutation: row r <-> original head dim d(r) such that
# rotate-half becomes the intra-quadrant +16 shuffle
def _dperm():
    d = np.zeros(DH, dtype=np.int64)
    for r in range(DH):
        qd, i = r // 32, r % 32
        if i < 16:
            d[r] = qd * 16 + i
        else:
            d[r] = 32 + qd * 16 + (i - 16)
    return d


def _rope_tables():
    dp = _dperm()
    inv = 1.0 / (10000.0 ** (np.arange(0, DH, 2, dtype=np.float32) / DH))
    t = np.arange(S, dtype=np.float32)
    ang = np.outer(inv, t)                               # [32, S] theta_j * s
    cos64 = np.cos(ang)[dp % 32, :]                      # [64, S] row-permuted
    sin64 = np.sin(ang)[dp % 32, :]
    sgn = np.where((np.arange(DH) % 32) < 16, -1.0, 1.0)[:, None]
    sinm64 = sin64 * sgn
    cosP = np.concatenate([cos64, cos64], 0).astype(BF_NP).copy()
    sinmP = np.concatenate([sinm64, sinm64], 0).astype(BF_NP).copy()
    return cosP, sinmP


def _pmaj(w):
    n = w.shape[1]
    return np.ascontiguousarray(
        w.reshape(8, 128, n).transpose(1, 0, 2)).astype(BF_NP)


def _mask2():
    r = np.arange(128)[:, None]
    c = np.arange(128)[None, :]
    m = (r <= c).astype(np.float32)
    return np.broadcast_to(m[:, None, :], (128, 2, 128)).astype(BF_NP).copy()


_NC_CACHE = {}


def _get_nc():
    if "nc" not in _NC_CACHE:
        _NC_CACHE["nc"] = build_bass()
    return _NC_CACHE["nc"]


def run(x, Wq, Wk, Wv, Wo, trace=False):
    nc = _get_nc()
    cosP, sinmP = _rope_tables()
    ident = np.eye(128, dtype=np.float32).astype(BF_NP)
    mask = _mask2()
    dp = _dperm()
    qperm = (np.arange(HPG)[:, None] * DH + dp[None, :]).reshape(-1)  # [256]
    in_maps = []
    for c in range(NCHIPS):
        b, g = c // KV, c % KV
        wk_g = Wk[:, g * DH:(g + 1) * DH][:, dp]         # row-permuted K
        wv_g = Wv[:, g * DH:(g + 1) * DH]                # V in natural order
        in_maps.append({
            "xcc": np.ascontiguousarray(
                x[b].T.reshape(8, 128, NCH, QB).transpose(2, 1, 0, 3)
            ).astype(BF_NP),
            "wq": _pmaj(Wq[:, g * DG:(g + 1) * DG][:, qperm]),
            "wkv": _pmaj(np.concatenate([wk_g, wv_g], axis=1)),
            "wo": np.ascontiguousarray(
                Wo[g * DG:(g + 1) * DG, :].reshape(2, 128, DM)
                .transpose(1, 0, 2)).astype(BF_NP),
            "cos2": cosP, "sinm2": sinmP,
            "ident": ident, "mask2": mask,
        })
    res = bass_utils.run_bass_kernel_spmd(
        nc, in_maps, core_ids=list(range(NCHIPS)), trace=trace)
    outs = [np.asarray(r["out"]).astype(np.float32) for r in res.results]
    full = np.zeros((B, S, DM), dtype=np.float32)
    for c in range(NCHIPS):
        full[c // KV] += outs[c]
    return full, res


def kernel(x, Wq, Wk, Wv, Wo):
    full, _ = run(np.asarray(x, dtype=np.float32), np.asarray(Wq),
                  np.asarray(Wk), np.asarray(Wv), np.asarray(Wo))
    return full


# revision 42
# speedup vs baseline: 1.1774x; 1.1774x over previous
"""GQA attention (B=2, S=2048, dm=1024, 16 Q heads / 4 KV heads, dh=64, RoPE,
causal) on 8 trn2 NeuronCores.

Sharding: core c = (b, g) with b = c // 4 (batch), g = c % 4 (KV group).
Each core computes its 4 Q heads + 1 KV head end-to-end (flash-style) plus its
partial Wo projection; host sums the 4 partials per batch element.

v4 (on top of v3's bf16 matmuls, transposed layouts, denominator-via-ones
rows, filler-interleaved projections):
  - K|V fused projection: one [128,(K|V)] stationary produces K^T and V^T
    in a single accumulation chain (halves the KV projection matmul time).
  - RoPE via DVE stream_shuffle: the head-dim order of Wq/Wk columns is
    permuted host-side so rotate-half becomes an intra-32-partition
    shuffle (+16 mod 32); signs folded into the sinm table. Rope is
    3 DVE ops + 1 GpSimd add, no ScalarE involvement.
  - exp table-set load triggered at t=0, before the bulk constant DMAs.
  - startup DMAs ki-interleaved across both HWDGE queues so the first
    projection matmuls start ~2us after DMA begins.
  - output leaves the core as bf16 (host upcasts), halving out-DMA.
  - tail: last-chunk denominator staging + half the final Wo casts run on
    the (by then idle) ScalarE instead of the DVE.
  - V tiles transposed by the DMA XBAR (SBUF->SBUF) instead of
    PE-transpose + PSUM staging + DVE copy: decongests the pa PSUM ring,
    which was the hidden source of run-to-run jitter and HAM re-throttles.

NOTE (measured on hw): fp8/DoubleRow for the P@V chain and Schraudolph
exp-on-DVE are numerically DEAD ENDS here -- attention output on iid random
data is a cancelling average, so per-prob noise (fp8 ~2.5-3.6% per element)
lands ~1:1 in the output norm-rel-err, busting the 2e-2 gate. The whole
P/V/score chain must stay bf16.
"""

import os
import sys
from collections import deque

import numpy as np
import ml_dtypes

try:
    from concourse import bass_utils
except ImportError:
    for _p in ("/opt/trn_rl_repo", "/root/.axon_site/_ro/trn_rl_repo"):
        if os.path.isdir(_p) and _p not in sys.path:
            sys.path.insert(0, _p)
    from concourse import bass_utils

import concourse.bass as bass
import concourse.mybir as mybir
import concourse.tile as tile
from concourse import bacc

F32 = mybir.dt.float32
BF = mybir.dt.bfloat16
EXP = mybir.ActivationFunctionType.Exp
COPY = mybir.ActivationFunctionType.Copy
MULT = mybir.AluOpType.mult
ADD = mybir.AluOpType.add

B, S, DM = 2, 2048, 1024
H, KV, DH = 16, 4, 64
HPG = H // KV          # 4 q-heads per kv group (per core)
DG = HPG * DH          # 256 local q dims
NCHIPS = 8
QB = 512               # q block width
KT = 128               # k tile width
NKT = S // KT          # 16 k tiles
NCH = S // QB          # 4 chunks == 4 q blocks

BF_NP = ml_dtypes.bfloat16

# rotate-half as intra-quadrant shuffle: out[i] = in[(i+16)%32] per 32-block
SHUF16 = [(i + 16) % 32 for i in range(32)]


def build_bass():
    nc = bacc.Bacc()
    xcc = nc.declare_dram_parameter("xcc", [NCH, 128, 8, QB], BF, isOutput=False)
    wq = nc.declare_dram_parameter("wq", [128, 8, DG], BF, isOutput=False)
    wkv = nc.declare_dram_parameter("wkv", [128, 8, 128], BF, isOutput=False)
    wo = nc.declare_dram_parameter("wo", [128, 2, DM], BF, isOutput=False)
    cos2 = nc.declare_dram_parameter("cos2", [128, S], BF, isOutput=False)
    sinm2 = nc.declare_dram_parameter("sinm2", [128, S], BF, isOutput=False)
    ident = nc.declare_dram_parameter("ident", [128, 128], BF, isOutput=False)
    mask2 = nc.declare_dram_parameter("mask2", [128, 2, 128], BF, isOutput=False)
    out = nc.declare_dram_parameter("out", [S, DM], BF, isOutput=True)

    from contextlib import ExitStack
    with tile.TileContext(nc) as tc, ExitStack() as es:
        cst = es.enter_context(tc.tile_pool(name="cst", bufs=1))
        sbQ = es.enter_context(tc.tile_pool(name="sbQ", bufs=1))
        sbX = es.enter_context(tc.tile_pool(name="sbX", bufs=2))
        sbR = es.enter_context(tc.tile_pool(name="sbR", bufs=3))
        sbP = es.enter_context(tc.tile_pool(name="sbP", bufs=6))
        sbN = es.enter_context(tc.tile_pool(name="sbN", bufs=3))
        sbO = es.enter_context(tc.tile_pool(name="sbO", bufs=4))
        ps = es.enter_context(tc.tile_pool(name="ps", bufs=2, space="PSUM"))

        xcs = [sbX.tile([128, 8, QB], BF, tag="xc", name=f"xc{i}")
               for i in range(NCH)]
        # trigger the exp table-set load (~2.7us) before the scalar queue
        # fills with bulk constant DMAs; input is a freshly-memset scratch
        wu_l = cst.tile([128, 128], BF)
        nc.gpsimd.memset(wu_l[:], 0.5)
        wu_act = cst.tile([128, 8], BF)
        nc.scalar.activation(wu_act[:], wu_l[:, 0:8], EXP, scale=0.01)
        # startup DMA: x pieces on the sync queue, weights + tables on the
        # scalar queue, ki-interleaved so the first projection matmuls can
        # start ~2us after DMA begins instead of waiting for 1.5MB
        wq_sb = cst.tile([128, 8, DG], BF)
        wkv_sb = cst.tile([128, 8, 128], BF)
        cos_sb = cst.tile([128, S], BF)
        sinm_sb = cst.tile([128, S], BF)
        nc.scalar.dma_start(wkv_sb[:], wkv[:])
        for k in range(4):
            nc.sync.dma_start(xcs[0][:, 2 * k:2 * k + 2, :],
                              xcc[0][:, 2 * k:2 * k + 2, :])
            nc.scalar.dma_start(wq_sb[:, 2 * k:2 * k + 2, :],
                                wq[:, 2 * k:2 * k + 2, :])
        nc.scalar.dma_start(cos_sb[:, 0:QB], cos2[:, 0:QB])
        nc.scalar.dma_start(sinm_sb[:, 0:QB], sinm2[:, 0:QB])
        nc.sync.dma_start(xcs[1][:], xcc[1][:])
        id_sb = cst.tile([128, 128], BF)
        nc.scalar.dma_start(id_sb[:], ident[:])
        mask_sb = cst.tile([128, 2, 128], BF)
        nc.scalar.dma_start(mask_sb[:], mask2[:])
        nc.sync.dma_start(xcs[2][:], xcc[2][:])
        nc.scalar.dma_start(cos_sb[:, QB:2 * QB], cos2[:, QB:2 * QB])
        nc.scalar.dma_start(sinm_sb[:, QB:2 * QB], sinm2[:, QB:2 * QB])
        nc.scalar.dma_start(cos_sb[:, 2 * QB:3 * QB], cos2[:, 2 * QB:3 * QB])
        nc.scalar.dma_start(sinm_sb[:, 2 * QB:3 * QB], sinm2[:, 2 * QB:3 * QB])
        wo_sb = cst.tile([128, 2, DM], BF)

        # HAM warm-up: keep the PE busy on scratch data while the first
        # DMAs land, so chunk-0 projections run at 2.4 GHz, not 1.2.
        wu_r = cst.tile([128, QB], BF)
        nc.gpsimd.memset(wu_r[:], 0.5)
        wu_ps = ps.tile([128, QB], F32, tag="cp", name="wu_ps")
        for _ in range(24):
            nc.tensor.matmul(wu_ps[:, 0:128], wu_l[:], wu_r[:, 0:128],
                             start=True, stop=True)

        # persistent activations
        qT2 = sbQ.tile([128, 2, S], BF)              # Q^T, head pairs stacked
        kT = sbQ.tile([128, S], BF)                  # K^T duplicated rows
        vt = [sbQ.tile([128, 128], BF, tag=f"v{i}", name=f"v{i}")
              for i in range(NKT)]
        for i in range(NKT):
            nc.gpsimd.memset(vt[i][:, DH:128], 1.0)  # denominator columns
        packed = sbQ.tile([128, 2, S], BF)           # normalized ctx^T

        def rope(dsts, qp, rows, c0):
            """dst (bf16) = qp[0:rows]*cos + shuffle16(qp[0:rows])*sinm.

            qp is fp32 PSUM; rows is 64 or 128 (1 or 2 stacked heads).
            Head-dim order is host-permuted so rotate-half is the intra-
            quadrant +16 shuffle with signs folded into sinm. DVE does
            shuffle + 2 multiplies, GpSimdE the final add.
            """
            rot = sbR.tile([128, QB], F32, tag="rot")
            nc.vector.stream_shuffle(rot[0:rows, :], qp[0:rows, :], SHUF16)
            t1 = sbR.tile([128, QB], BF, tag="t1")
            nc.vector.tensor_tensor(t1[0:rows, :], qp[0:rows, :],
                                    cos_sb[0:rows, c0:c0 + QB], MULT)
            rs = sbR.tile([128, QB], BF, tag="rs")
            nc.vector.tensor_tensor(rs[0:rows, :], rot[0:rows, :],
                                    sinm_sb[0:rows, c0:c0 + QB], MULT)
            for d, lo, nr in dsts:
                nc.gpsimd.tensor_tensor(
                    d, t1[lo:lo + nr, :], rs[lo:lo + nr, :], ADD)

        def proj_fillers(ch):
            """Closures emitting the projections+rope for chunk ch."""
            c0 = ch * QB
            xc = xcs[ch]
            fl = []
            state = {}

            def qchain(mt, lo, hi):
                def go():
                    if lo == 0:
                        state[mt] = ps.tile([128, QB], F32, tag="pa",
                                            name=f"qp{ch}{mt}", uniquify=True)
                    qp = state[mt]
                    for ki in range(lo, hi):
                        nc.tensor.matmul(
                            qp[:], wq_sb[:, ki, mt * 128:mt * 128 + 128],
                            xc[:, ki, :], start=(ki == 0), stop=(ki == 7))
                return go

            def qrope(mt):
                def go():
                    rope([(qT2[:, mt, c0:c0 + QB], 0, 128)], state[mt], 128, c0)
                return go

            for lo in range(0, 8, 2):
                fl.append(qchain(0, lo, min(lo + 2, 8)))
            fl.append(qrope(0))

            def kvchain(lo, hi):
                def go():
                    if lo == 0:
                        state['kv'] = ps.tile([128, QB], F32, tag="pa",
                                              name=f"kvp{ch}", uniquify=True)
                    kvp = state['kv']
                    for ki in range(lo, hi):
                        nc.tensor.matmul(kvp[:], wkv_sb[:, ki, :],
                                         xc[:, ki, :],
                                         start=(ki == 0), stop=(ki == 7))
                return go

            for lo in range(0, 8, 2):
                fl.append(kvchain(lo, min(lo + 2, 8)))

            def krope():
                rope([(kT[0:DH, c0:c0 + QB], 0, DH)], state['kv'], DH, c0)
                # duplicate K^T rows for the second concurrent head matmul
                nc.sync.dma_start(kT[DH:128, c0:c0 + QB], kT[0:DH, c0:c0 + QB])
            fl.append(krope)

            def vmove():
                vTs = sbR.tile([DH, QB], BF, tag="vT")
                nc.vector.tensor_copy(vTs[:], state['kv'][DH:128, :])
                state['vT'] = vTs
            fl.append(vmove)

            def vtrans(sub):
                def go():
                    # DMA XBAR transpose SBUF->SBUF: no PE pass, no DVE copy,
                    # and no tp allocation in the contended pa PSUM ring
                    nc.sync.dma_start(
                        vt[ch * 4 + sub][:, 0:DH],
                        state['vT'][:, sub * 128:sub * 128 + 128],
                        transpose=True)
                return go

            for sub in range(QB // 128):
                fl.append(vtrans(sub))
            for lo in range(0, 8, 2):
                fl.append(qchain(1, lo, min(lo + 2, 8)))
            fl.append(qrope(1))
            return fl

        def wo_fillers(jb, tail=False):
            """Closures emitting the Wo projection for q block jb. In the
            tail, ScalarE is idle: alternate the PSUM->SBUF casts onto it so
            the DVE does not serialize the endgame."""
            q0 = jb * QB
            fl = []

            def wostep(st, nb):
                def go():
                    s0 = q0 + st * 128
                    op = ps.tile([128, QB], F32, tag="pa",
                                 name=f"op{jb}{st}{nb}", uniquify=True)
                    for kt2 in range(2):
                        nc.tensor.matmul(
                            op[:], packed[:, kt2, s0:s0 + 128],
                            wo_sb[:, kt2, nb * QB:nb * QB + QB],
                            start=(kt2 == 0), stop=(kt2 == 1))
                    ot = sbO.tile([128, QB], BF, tag="ot")
                    if tail and (st + nb) % 2 == 0:
                        nc.scalar.activation(ot[:], op[:], COPY)
                    else:
                        nc.vector.tensor_copy(ot[:], op[:])
                    # tail: drain the last block's output on both queues
                    dq = nc.scalar if tail and nb == 1 else nc.sync
                    dq.dma_start(
                        out[s0:s0 + 128, nb * QB:nb * QB + QB], ot[:])
                return go

            for st in range(QB // 128):
                for nb in range(2):
                    fl.append(wostep(st, nb))
            return fl

        fillers = deque()          # entries: (deadline_chunk_or_None, fn)
        dummy_budget = [32]

        def drain(n):
            ran = False
            for _ in range(n):
                if not fillers:
                    break
                fillers.popleft()[1]()
                ran = True
            if not ran and dummy_budget[0] > 0:
                # keep-warm: short scratch matmul so the HAM clock gate does
                # not re-throttle during scalar-paced stretches
                dummy_budget[0] -= 1
                dp = ps.tile([128, QB], F32, tag="pa", name="dum",
                             uniquify=True)
                nc.tensor.matmul(dp[:, 0:256], wu_l[:], wu_r[:, 0:256],
                                 start=True, stop=True)

        # chunks 0 and 1 project upfront: DVE/GpSimdE are idle here, so
        # their rope chains cost nothing and attention never waits on them
        for f in proj_fillers(0):
            f()
        for f in proj_fillers(1):
            f()

        for ch in range(NCH):
            jb = ch
            q0 = jb * QB
            nkt = 4 * (jb + 1)
            # just-in-time bulk DMAs, then queue projections + previous Wo
            if ch == 0:
                nc.scalar.dma_start(wo_sb[:], wo[:])
                nc.sync.dma_start(xcs[3][:], xcc[3][:])
                nc.scalar.dma_start(cos_sb[:, 3 * QB:], cos2[:, 3 * QB:])
                nc.scalar.dma_start(sinm_sb[:, 3 * QB:], sinm2[:, 3 * QB:])
            # force only overdue projections (normally none: the per-step
            # drain spreads them); never burst the whole queue
            while fillers and fillers[0][0] is not None and fillers[0][0] <= ch:
                fillers.popleft()[1]()
            if ch + 2 < NCH:
                fillers.extend((ch + 2, f) for f in proj_fillers(ch + 2))
            if ch >= 1:
                fillers.extend((None, f) for f in wo_fillers(ch - 1))
            steps_left = 2 * nkt
            prefetch = {}
            for pair in range(2):
                cps = [ps.tile([128, QB], F32, tag="cp", name=f"cp{jb}{pair}{hh}")
                       for hh in range(2)]

                def score_step(kt_i):
                    """scores + exp + mask for one k tile (one step ahead of
                    the ctx matmuls so the PE never sits behind ScalarE)."""
                    cc = max(0, (kt_i - 4 * jb) * KT)
                    sp = ps.tile([128, 2, QB], F32, tag="sp", name="sp",
                                 uniquify=True)
                    for hh in range(2):
                        p0 = hh * DH
                        nc.tensor.matmul(
                            sp[:, hh, cc:],
                            kT[p0:p0 + DH, kt_i * KT:kt_i * KT + KT],
                            qT2[p0:p0 + DH, pair, q0 + cc:q0 + QB],
                            start=True, stop=True)
                    pt = sbP.tile([128, 2, QB], BF, tag="pt", name="pt",
                                  uniquify=True)
                    nc.scalar.activation(pt[:, :, cc:], sp[:, :, cc:], EXP,
                                         scale=0.125)
                    if kt_i >= 4 * jb:
                        nc.vector.tensor_tensor(
                            pt[:, :, cc:cc + KT], pt[:, :, cc:cc + KT],
                            mask_sb[:], MULT)
                    return pt

                pts = dict(prefetch)
                prefetch.clear()
                if 0 not in pts:
                    pts[0] = score_step(0)
                for kt_i in range(nkt):
                    if kt_i + 1 < nkt and kt_i + 1 not in pts:
                        pts[kt_i + 1] = score_step(kt_i + 1)
                    cc = max(0, (kt_i - 4 * jb) * KT)
                    pt = pts.pop(kt_i)
                    for hh in range(2):
                        nc.tensor.matmul(
                            cps[hh][:, cc:], vt[kt_i][:], pt[:, hh, cc:],
                            start=(kt_i == 0), stop=(kt_i == nkt - 1))
                    # keep the PE busy while ScalarE runs exp
                    drain(min(2, -(-len(fillers) // max(1, steps_left))))
                    steps_left -= 1
                if pair == 1:
                    # flush fillers first: their rope/copy ops must not queue
                    # behind the normalize chain on the DVE
                    drain(len(fillers))
                elif ch == NCH - 1:
                    # last chunk: issue pair 1's first scores+exps before the
                    # normalize chain occupies the engine queues
                    pair = 1
                    prefetch[0] = score_step(0)
                    prefetch[1] = score_step(1)
                    pair = 0
                # normalize: 1/denom (rows 64:128) * ctx (rows 0:64);
                # in the tail ScalarE is idle, so stage denominators there
                for hh in range(2):
                    h = 2 * pair + hh
                    dns = sbN.tile([DH, QB], F32, tag="dns")
                    if ch == NCH - 1:
                        nc.scalar.activation(dns[:], cps[hh][DH:128, :], COPY)
                    else:
                        nc.vector.tensor_copy(dns[:], cps[hh][DH:128, :])
                    rb = sbN.tile([DH, QB], F32, tag="rb")
                    nc.vector.reciprocal_approx_fast(rb[:], dns[:])
                    p0 = (h % 2) * DH
                    nc.vector.tensor_tensor(
                        packed[p0:p0 + DH, h // 2, q0:q0 + QB],
                        cps[hh][0:DH, :], rb[:], MULT)
            drain(len(fillers))
        for f in wo_fillers(NCH - 1, tail=True):
            f()
    nc.compile()
    return nc


# within-head row permutation: row r <-> original head dim d(r) such that
# rotate-half becomes the intra-quadrant +16 shuffle
def _dperm():
    d = np.zeros(DH, dtype=np.int64)
    for r in range(DH):
        qd, i = r // 32, r % 32
        if i < 16:
            d[r] = qd * 16 + i
        else:
            d[r] = 32 + qd * 16 + (i - 16)
    return d


def _rope_tables():
    dp = _dperm()
    inv = 1.0 / (10000.0 ** (np.arange(0, DH, 2, dtype=np.float32) / DH))
    t = np.arange(S, dtype=np.float32)
    ang = np.outer(inv, t)                               # [32, S] theta_j * s
    cos64 = np.cos(ang)[dp % 32, :]                      # [64, S] row-permuted
    sin64 = np.sin(ang)[dp % 32, :]
    sgn = np.where((np.arange(DH) % 32) < 16, -1.0, 1.0)[:, None]
    sinm64 = sin64 * sgn
    cosP = np.concatenate([cos64, cos64], 0).astype(BF_NP).copy()
    sinmP = np.concatenate([sinm64, sinm64], 0).astype(BF_NP).copy()
    return cosP, sinmP


def _pmaj(w):
    n = w.shape[1]
    return np.ascontiguousarray(
        w.reshape(8, 128, n).transpose(1, 0, 2)).astype(BF_NP)


def _mask2():
    r = np.arange(128)[:, None]
    c = np.arange(128)[None, :]
    m = (r <= c).astype(np.float32)
    return np.broadcast_to(m[:, None, :], (128, 2, 128)).astype(BF_NP).copy()


_NC_CACHE = {}


def _get_nc():
    if "nc" not in _NC_CACHE:
        _NC_CACHE["nc"] = build_bass()
    return _NC_CACHE["nc"]


def run(x, Wq, Wk, Wv, Wo, trace=False):
    nc = _get_nc()
    cosP, sinmP = _rope_tables()
    ident = np.eye(128, dtype=np.float32).astype(BF_NP)
    mask = _mask2()
    dp = _dperm()
    qperm = (np.arange(HPG)[:, None] * DH + dp[None, :]).reshape(-1)  # [256]
    in_maps = []
    for c in range(NCHIPS):
        b, g = c // KV, c % KV
        wk_g = Wk[:, g * DH:(g + 1) * DH][:, dp]         # row-permuted K
        wv_g = Wv[:, g * DH:(g + 1) * DH]                # V in natural order
        in_maps.append({
            "xcc": np.ascontiguousarray(
                x[b].T.reshape(8, 128, NCH, QB).transpose(2, 1, 0, 3)
            ).astype(BF_NP),
            "wq": _pmaj(Wq[:, g * DG:(g + 1) * DG][:, qperm]),
            "wkv": _pmaj(np.concatenate([wk_g, wv_g], axis=1)),
            "wo": np.ascontiguousarray(
                Wo[g * DG:(g + 1) * DG, :].reshape(2, 128, DM)
                .transpose(1, 0, 2)).astype(BF_NP),
            "cos2": cosP, "sinm2": sinmP,
            "ident": ident, "mask2": mask,
        })
    res = bass_utils.run_bass_kernel_spmd(
        nc, in_maps, core_ids=list(range(NCHIPS)), trace=trace)
    outs = [np.asarray(r["out"]).astype(np.float32) for r in res.results]
    full = np.zeros((B, S, DM), dtype=np.float32)
    for c in range(NCHIPS):
        full[c // KV] += outs[c]
    return full, res


def kernel(x, Wq, Wk, Wv, Wo):
    full, _ = run(np.asarray(x, dtype=np.float32), np.asarray(Wq),
                  np.asarray(Wk), np.asarray(Wv), np.asarray(Wo))
    return full
